# revision 1
# baseline (speedup 1.0000x reference)
"""Trainium2 Bass kernel for nn_CWTLayer: CWT (mexh, 128 scales) + bilinear
resize (B,128,4096,C) -> (B,224,224,C).

Strategy: the whole pipeline (reflect-pad -> per-scale conv with
flip(gather(int_psi)) -> first difference -> center slice -> bilinear time
resize 4096->224) is linear and time-invariant per scale, and the time resize
has exactly 7 fractional phases (4096/224 = 128/7). So conv+diff+slice+time-
resize fold into one banded matrix per phase: for output column w = 7q+p,

    out[s, r, w] = dot(xp[r, base_p + 128 q :  + Lp], Wp[:, s])

with two edge columns (w=0, w=223) using their own matrices (resize-weight
truncation + renormalization at the boundary). The scale resize 128->224 is a
final small matmul. Device work per core = ~260 accumulating float32r matmuls
(K=128 chunks, N=384) + the scale-resize matmuls; ~51.5us/core modeled.

Sharding: data-parallel over batch B (32 = 8 cores x 4), int_psi-derived
weights replicated. Host does only input layout (pad/transpose) and the tiny
O(sum Ks) weight-matrix construction from int_psi_base.
"""

import numpy as np

# ---------------- static configuration ----------------
B, T, C = 32, 4096, 3
SCALES = 128
KLF = 10
OUT_H, OUT_W = 224, 224
N_PSI = 4096
SPAN = 16.0
STEP = SPAN / (N_PSI - 1)
MAX_SCALE = T // (2 * KLF)
SCALES_VEC = np.logspace(np.log10(2.0), np.log10(MAX_SCALE), SCALES).astype(np.float32)
PAD = min((N_PSI - 1) // 2, T - 1)          # 2047
TP = T + 2 * PAD                            # 8190
TP2 = -(-TP // 128) * 128                   # 8192
AWORDS = TP2 // 128                         # 64
NPHASE = 7
QN = OUT_W // NPHASE                        # 32
N_CORES = 8
BPC = B // N_CORES                          # 4 batches per core
RPC = C * BPC                               # 12 rows per core

J_LIST = []
for _s in SCALES_VEC:
    _sf = float(_s)
    _n = int(np.ceil(_sf * SPAN + 1.0))
    _j = np.floor(np.arange(_n, dtype=np.float64) / (_sf * STEP)).astype(np.int32)
    J_LIST.append(np.clip(_j, 0, N_PSI - 1))
KS = [len(j) for j in J_LIST]
DS = [max((TP - k - T) // 2, 0) for k in KS]


def _resize_weight_mat(in_size, out_size):
    """jax.image.resize bilinear (antialias=True) weight matrix, float64."""
    scale = out_size / in_size
    inv_scale = 1.0 / scale
    kernel_scale = max(inv_scale, 1.0)
    sample_f = (np.arange(out_size, dtype=np.float64) + 0.5) * inv_scale - 0.5
    x = np.abs(sample_f[:, None] - np.arange(in_size, dtype=np.float64)[None, :]) / kernel_scale
    w = np.maximum(0.0, 1.0 - x)
    total = w.sum(axis=1, keepdims=True)
    w = np.where(np.abs(total) > 1000.0 * np.finfo(np.float32).eps, w / total, 0.0)
    ok = (sample_f >= -0.5) & (sample_f <= in_size - 0.5)
    return np.where(ok[:, None], w, 0.0)


_WT = _resize_weight_mat(T, OUT_W)          # (224, 4096)
_A = _resize_weight_mat(SCALES, OUT_H)      # (224, 128)

# Static geometry of the weight groups: 9 kernel-groups (7 phases + 2 edge
# columns) x 2 scale-ranges (short-kernel scales 0..SPLIT-1 need far fewer
# 128-chunks than long-kernel scales SPLIT..127, so splitting nearly halves
# the weight bytes shipped). Geometry depends only on shapes, not psi values.
# NB: engines only accept start partitions at multiples of 32, so SPLIT
# must be 32-aligned (the B-group copy writes ot[SPLIT:128]).
# Splitting a group saves ~0.76MB of weight DMA but costs ~4.6 extra PE
# chunks (each chunk is N=384 cycles regardless of group width). With PE
# the binding resource and DMA idle after the preload, only N_SPLIT_PHASES
# phases are split; edge groups always split (their chunks are N=12, so the
# extra chunk is ~free).
SPLIT = 64
N_SPLIT_PHASES = 4


def _kernel_rows():
    rows = []
    for p in range(NPHASE):
        wrow = p if p != 0 else 7
        shift = 128 if p == 0 else 0
        sr = (((0, SPLIT), (SPLIT, SCALES)) if p < N_SPLIT_PHASES
              else ((0, SCALES),))
        rows.append(dict(kind="phase", p=p, w_row=wrow, shift=shift,
                         sranges=sr))
    for wrow in (0, OUT_W - 1):
        rows.append(dict(kind="edge", p=None, w_row=wrow, shift=0,
                         sranges=((0, SPLIT), (SPLIT, SCALES))))
    return rows


_KROWS = _kernel_rows()
_WGROUPS = []
_acc_cols = 0
for _ki, _kr in enumerate(_KROWS):
    _t_nz = np.nonzero(_WT[_kr["w_row"]])[0]
    _t0, _t1 = int(_t_nz[0]), int(_t_nz[-1])
    for _s0, _s1 in _kr["sranges"]:
        _lo = min(_t0 + DS[s] - _kr["shift"] for s in range(_s0, _s1))
        _hi = max(_t1 + DS[s] - _kr["shift"] + KS[s] + 1 for s in range(_s0, _s1))
        _base = (_lo // 128) * 128
        _nck = -(-(_hi - _base) // 128)
        _g = {k: v for k, v in _kr.items() if k != "sranges"}
        _WGROUPS.append(dict(**_g, ki=_ki, t0=_t0, t1=_t1, s0=_s0, s1=_s1,
                             base=_base, nck=_nck, goff=_acc_cols))
        _acc_cols += _nck * (_s1 - _s0)
_FTOT = _acc_cols


def _build_wall(int_psi_base):
    """Fill Wall (128, FTOT) float32: group wg chunk c occupies columns
    [goff + c*W : goff + (c+1)*W], W = s1-s0; Wall[kk, goff + c*W + (s-s0)]
    = folded kernel value at u = base + 128c + kk for scale s."""
    psi = np.asarray(int_psi_base, dtype=np.float64)
    wall = np.zeros((128, _FTOT), dtype=np.float64)
    for g in _WGROUPS:
        wt_row = _WT[g["w_row"], g["t0"]: g["t1"] + 1]
        W = g["s1"] - g["s0"]
        col = np.zeros((g["nck"] * 128,), dtype=np.float64)
        for sidx in range(g["s0"], g["s1"]):
            kern = psi[J_LIST[sidx]][::-1]
            kpad = np.concatenate([[0.0], kern, [0.0]])
            dk = -np.sqrt(np.float64(SCALES_VEC[sidx])) * (kpad[:-1] - kpad[1:])
            h = np.convolve(wt_row, dk)
            st = g["t0"] + DS[sidx] - g["shift"] - g["base"]
            col[:] = 0.0
            col[st: st + len(h)] = h
            cv = col.reshape(g["nck"], 128)            # [c, kk]
            j0 = g["goff"] + (sidx - g["s0"])
            wall[:, j0: j0 + g["nck"] * W: W] = cv.T
    return np.ascontiguousarray(wall.astype(np.float32))


def _pad_rows(x):
    xp = np.pad(x, ((0, 0), (PAD, PAD), (0, 0)), mode="reflect")
    xp = np.transpose(xp, (2, 0, 1)).reshape(C * B, TP)
    out = np.zeros((C * B, TP2), dtype=np.float32)
    out[:, :TP] = xp
    return out


# ---------------- bass program ----------------
_NC_CACHE = {}


def _get_nc():
    if "nc" in _NC_CACHE:
        return _NC_CACHE["nc"]
    import concourse.bacc as bacc
    import concourse.mybir as mybir
    from concourse import tile

    f32 = mybir.dt.float32
    f32r = mybir.dt.float32r
    nc = bacc.Bacc(None)
    xpt_d = nc.dram_tensor("xpt", [128, RPC * AWORDS], f32r, kind="ExternalInput")
    wall_d = nc.dram_tensor("wall", [128, _FTOT], f32r, kind="ExternalInput")
    at_d = nc.dram_tensor("at", [128, OUT_H], f32r, kind="ExternalInput")
    out_d = nc.dram_tensor("out", [BPC, OUT_H, OUT_W, C], f32, kind="ExternalOutput")

    with tile.TileContext(nc) as tc:
        with (
            tc.tile_pool(name="const", bufs=1) as cpool,
            tc.tile_pool(name="w", bufs=2) as wpool,
            tc.tile_pool(name="ot", bufs=1) as otpool,
            tc.tile_pool(name="psa", bufs=1, space="PSUM") as psapool,
            tc.tile_pool(name="psb", bufs=2, space="PSUM") as psbpool,
            tc.tile_pool(name="ps2", bufs=5, space="PSUM") as ps2pool,
            tc.tile_pool(name="res", bufs=1) as respool,
        ):
            xpt = cpool.tile([128, RPC * AWORDS], f32r)
            nc.sync.dma_start(out=xpt[:], in_=xpt_d[:])

            ot = otpool.tile([128, RPC * OUT_W], f32r)
            xpr = xpt[:].rearrange("p (r a) -> p r a", r=RPC)
            # w index inside ot free dim: r*224 + w ; w = 7q + s
            ot_phase = ot[:].rearrange("p (r q s) -> p r s q", q=QN, s=NPHASE)
            ot_rw = ot[:].rearrange("p (r w) -> p r w", r=RPC)

            # Preload every weight tile upfront: transfers stream back-to-back
            # on the DMA engines from t=0 instead of being gated by tile-slot
            # reuse, so PE never waits at a group boundary.
            wt_tiles = []
            for gi, g in enumerate(_WGROUPS):
                nck, W = g["nck"], g["s1"] - g["s0"]
                wt = wpool.tile([128, nck * W], f32r, tag=f"w{gi}")
                nc.sync.dma_start(
                    out=wt[:], in_=wall_d[:, g["goff"]: g["goff"] + nck * W])
                wt_tiles.append(wt)
            # at is only needed by the scale-resize at the end
            at = cpool.tile([128, OUT_H], f32r)
            nc.sync.dma_start(out=at[:], in_=at_d[:])

            ncopy = 0
            for ki in range(len(_KROWS)):
                subgroups = [(i, g) for i, g in enumerate(_WGROUPS)
                             if g["ki"] == ki]
                nfree = QN if subgroups[0][1]["kind"] == "phase" else 1
                for gi, g in subgroups:
                    nck, W = g["nck"], g["s1"] - g["s0"]
                    base_a = g["base"] // 128
                    wt = wt_tiles[gi]
                    # own PSUM tile at partition base 0 (the matmul ISA
                    # requires dst partition base 0); the scale offset is
                    # applied by the PSUM->SBUF copy's dst partition instead.
                    # Full-width (dense) groups share the psb slots.
                    pool = psapool if (W == SPLIT and g["s0"] == 0) else psbpool
                    ps = pool.tile([W, RPC * nfree], f32,
                                   tag="ps0" if pool is psapool else "ps1")
                    for c in range(nck):
                        if nfree > 1:
                            rhs = xpr[:, :, base_a + c: base_a + c + nfree]
                        else:
                            # singleton free dim dropped: fp32r needs an even
                            # innermost count on the moving operand
                            rhs = xpr[:, :, base_a + c]
                        nc.tensor.matmul(
                            ps[:], wt[:, c * W:(c + 1) * W], rhs,
                            start=(c == 0), stop=(c == nck - 1))
                    if g["kind"] == "phase":
                        dst = ot_phase[g["s0"]: g["s1"], :, g["p"], :]
                        psv = ps[:].rearrange("p (r q) -> p r q", r=RPC)
                    else:
                        dst = ot_rw[g["s0"]: g["s1"], :, g["w_row"]]
                        psv = ps[:]
                    if ncopy % 2 == 0:
                        nc.vector.tensor_copy(dst, psv)
                    else:
                        nc.scalar.copy(dst, psv)
                    ncopy += 1

            # scale resize 128 -> 224: contiguous 2-row (448-col) rhs slices
            # keep fp32r's even-innermost-count rule; the (c,b,w)->(b,w,c)
            # interleave happens in the strided PSUM->SBUF copies.
            out_hb = out_d[:].rearrange("b h w c -> h b (w c)")
            RR = 2                                           # OT rows per matmul
            NCH = RR * OUT_W                                 # 448 cols
            # j pairs (r=2j, 2j+1) cover batches (b, b+1) for one channel; the
            # order [0,2,4, 1,3,5] completes batch pair (0,1) after three
            # matmuls so its output DMA overlaps the remaining resize work.
            nres = 0
            for hb, hm in ((0, 128), (1, OUT_H - 128)):
                res = respool.tile([128, BPC * OUT_W * C], f32, tag=f"res{hb}")
                res_v = res[:].rearrange("p (b w c) -> p b w c", b=BPC, w=OUT_W)
                for ji, j in enumerate((0, 2, 4, 1, 3, 5)):
                    rhs = ot[:, j * NCH:(j + 1) * NCH]       # [p][448]
                    ps2 = ps2pool.tile([128, NCH], f32, tag="ps2")
                    nc.tensor.matmul(
                        ps2[:hm, :], at[:, hb * 128: hb * 128 + hm],
                        rhs, start=True, stop=True)
                    for rr in range(RR):
                        r = j * RR + rr
                        cc, b = divmod(r, BPC)
                        dst = res_v[:hm, b, :, cc]           # [p][w:224] stride 3
                        src = ps2[:hm, rr * OUT_W:(rr + 1) * OUT_W]
                        # DVE is ~2x faster than ACT for these copies:
                        # give it two of every three
                        if nres % 3 == 2:
                            nc.scalar.copy(dst, src)
                        else:
                            nc.vector.tensor_copy(dst, src)
                        nres += 1
                    if ji == 2 or ji == 5:
                        b0 = 0 if ji == 2 else 2
                        nc.sync.dma_start(
                            out=out_hb[hb * 128: hb * 128 + hm, b0:b0 + 2, :],
                            in_=res[:hm, b0 * OUT_W * C:(b0 + 2) * OUT_W * C]
                            .rearrange("p (b f) -> p b f", b=2))

    nc.finalize()
    _NC_CACHE["nc"] = nc
    return nc


def _prepare_in_maps(x, int_psi_base):
    x = np.asarray(x, dtype=np.float32)
    wall = _build_wall(int_psi_base)
    atm = np.ascontiguousarray(_A.T.astype(np.float32))      # (128, 224)
    xp = _pad_rows(x)                                        # (96, 8192)

    in_maps = []
    for core in range(N_CORES):
        rows = [c * B + core * BPC + bl for c in range(C) for bl in range(BPC)]
        xpc = xp[rows]                                       # (12, 8192)
        xpt = np.ascontiguousarray(
            xpc.reshape(RPC, AWORDS, 128).transpose(2, 0, 1).reshape(128, RPC * AWORDS))
        in_maps.append({"xpt": xpt, "wall": wall, "at": atm})
    return in_maps


def _run(x, int_psi_base, **spmd_kwargs):
    from concourse.bass_utils import run_bass_kernel_spmd

    in_maps = _prepare_in_maps(x, int_psi_base)
    nc = _get_nc()
    res = run_bass_kernel_spmd(nc, in_maps, list(range(N_CORES)), **spmd_kwargs)
    out = np.concatenate([res.results[i]["out"] for i in range(N_CORES)], axis=0)
    return out, res


def kernel(x, int_psi_base):
    return _run(x, int_psi_base)[0]

